# revision 26
# baseline (speedup 1.0000x reference)
"""Trainium2 Bass kernel: windowed 32-pt FFT -> top-8 magnitude mask -> iFFT.

Pure data parallel over batch (8 cores x 131072 rows). Per core, 64 tiles of
[128, 512] in freq-major layout (partition 32g+n = half-spectrum component n
of row-group g; column f = row 512g+f of the tile), processed in 32 pairs so
every vector-engine pass runs at [128, 1024] (halves per-instruction setup).

Per-pair pipeline:
  1. one DMA in: bf16 hi/lo split of both tiles [128, 2048].
  2. PE: g = Bh.T@xh + Bh.T@xl + Bl.T@xh per tile (bf16 matmuls, PSUM
     accumulate) -- windowed half-spectrum DFT (Re_0..16, Im_1..15 per
     32-group), fp32-grade precision from the bf16x2 split.
  3. ACT: sq = g^2 (PSUM->SBUF fp32) per tile.
  4. PE: s_rm = sq_chunk.T @ Pm_all per 128-column chunk (fp32): pair-sum
     Re^2+Im^2, the (1 - k*2^-20) tie-break bias, AND the row-major
     transpose fused into one matmul whose moving operand is 128 wide.
  5. ACT: mag_rm = sqrt(s_rm) (PSUM->SBUF fp32) -- biased magnitudes;
     partition = row, 32 segments of 32 freqs per pair.
  6. DVE: InstMax per segment -> sorted top-8; entry 7 = row threshold.
  7. DVE: mask = (mag_rm >= th broadcast); Pool: coef_rm = mask * mag_rm
     (bf16 out).
  8. XBAR DMA transpose: all eight [128,128] chunks of coef_rm back to
     freq-major in one instruction, straight to SBUF (bit-exact; replaces
     a PE transpose pass plus an ACT evacuation pass).
  9. PE: rec = Cm.T @ coef (bf16, one matmul per PSUM bank).
 10. ACT: evac rec -> SBUF bf16; DMA out (host converts to fp32).

The work is emitted as a four-stage software pipeline skewed across pairs
(front: DMA/DFT/square/pair-sum; midA: sqrt/top-8; midB: mask/mul;
back: transpose/iFFT/evac/out) so the in-order engine queues never block
an early-stage pass behind a late-stage pass of a previous pair.

The multiplicative bias makes conjugate-pair magnitudes strictly decreasing
in k, so ">= 8th largest" selects exactly 8 with ties broken toward lower k
like jax.lax.top_k; the cosine basis is symmetric under k -> 32-k so the
pair choice cannot change the output.
"""

import math

import numpy as np

B_TOTAL = 1048576
S = 32
N_CORES = 8
R_PER_CORE = B_TOTAL // N_CORES  # 131072
TILE_F = 512                     # rows per 32-partition group per tile
ROWS_PER_TILE = 4 * TILE_F       # 2048
N_TILES = R_PER_CORE // ROWS_PER_TILE  # 64
N_PAIRS = N_TILES // 2
ETA = 2.0 ** -20

_cache = {}


def _bf16(a):
    import ml_dtypes
    return a.astype(np.float32).astype(ml_dtypes.bfloat16)


def _build_consts():
    import ml_dtypes

    n = np.arange(S, dtype=np.float64)
    w = (0.5 - 0.5 * np.cos(2.0 * np.pi * np.arange(S, dtype=np.float32) / S))
    w = w.astype(np.float32).astype(np.float64)  # fp32 window values

    B32 = np.zeros((S, S), dtype=np.float64)
    for m in range(17):
        B32[:, m] = w * np.cos(2.0 * np.pi * m * n / S)
    for j in range(1, 16):
        B32[:, 16 + j] = -w * np.sin(2.0 * np.pi * j * n / S)
    B32f = B32.astype(np.float32)

    c = 1.0 - np.arange(S, dtype=np.float64) * ETA

    # Pair-sum + reflect + bias matrix: s_rm[., k] = c_k * (sq_Rej + sq_Imj)
    Pm = np.zeros((S, S), dtype=np.float64)
    for kk in range(S):
        j = min(kk, S - kk)
        Pm[j, kk] = c[kk]
        if 1 <= j <= 15:
            Pm[16 + j, kk] = c[kk]

    Cm = np.zeros((S, S), dtype=np.float64)
    for kk in range(S):
        Cm[kk, :] = np.cos(2.0 * np.pi * kk * n / S) / (S * math.sqrt(c[kk]))

    def blockdiag4(M, dtype):
        out = np.zeros((128, 128), dtype=dtype)
        for g in range(4):
            out[g * 32:(g + 1) * 32, g * 32:(g + 1) * 32] = M.astype(dtype)
        return out

    bh_s = _bf16(B32f)
    bl_s = _bf16(B32f - bh_s.astype(np.float32))
    bh = blockdiag4(bh_s, ml_dtypes.bfloat16)
    bl = blockdiag4(bl_s, ml_dtypes.bfloat16)
    pm = blockdiag4(Pm, np.float32)
    cm = blockdiag4(Cm, ml_dtypes.bfloat16)
    eye = np.eye(128, dtype=ml_dtypes.bfloat16)
    return bh, bl, pm, cm, eye


def _build_program(repeat: int = 1):
    import concourse.mybir as mybir
    from concourse import bacc
    from concourse.tile import TileContext

    f32 = mybir.dt.float32
    bf16 = mybir.dt.bfloat16
    nc = bacc.Bacc("TRN2", target_bir_lowering=False, debug=False)

    W = 2 * TILE_F            # 1024: pair width in rows
    x_d = nc.dram_tensor("x", [N_TILES, 128, W], bf16, kind="ExternalInput")
    bh_d = nc.dram_tensor("Bh", [128, 128], bf16, kind="ExternalInput")
    bl_d = nc.dram_tensor("Bl", [128, 128], bf16, kind="ExternalInput")
    pm_d = nc.dram_tensor("Pm", [128, 128], f32, kind="ExternalInput")
    cm_d = nc.dram_tensor("Cm", [128, 128], bf16, kind="ExternalInput")
    eye_d = nc.dram_tensor("Eye", [128, 128], bf16, kind="ExternalInput")
    out_d = nc.dram_tensor("out", [N_TILES, 128, TILE_F], bf16,
                           kind="ExternalOutput")

    x_v = x_d.ap()
    out_v = out_d.ap()
    SEGS = W // 32  # 32 segments of 32 per pair

    with TileContext(nc) as tc:
        with (
            tc.tile_pool(name="consts", bufs=1) as cpool,
            tc.tile_pool(name="io", bufs=4) as io_pool,
            tc.tile_pool(name="work", bufs=4) as work_pool,
            # PSUM budget (8 banks of [128,512]f32): g-pair 2 + s-pair 2x2 + rec-pair 2
            tc.tile_pool(name="ps_g", bufs=1, space="PSUM") as ps_g,
            tc.tile_pool(name="ps_s", bufs=2, space="PSUM") as ps_s,
            tc.tile_pool(name="ps_r", bufs=1, space="PSUM") as ps_r,
        ):
            bh = cpool.tile([128, 128], bf16, tag="bh")
            bl = cpool.tile([128, 128], bf16, tag="bl")
            pm = cpool.tile([128, 128], f32, tag="pm")
            cm = cpool.tile([128, 128], bf16, tag="cm")
            eye = cpool.tile([128, 128], bf16, tag="eye")
            nc.sync.dma_start(bh[:], bh_d.ap())
            nc.sync.dma_start(bl[:], bl_d.ap())
            nc.sync.dma_start(pm[:], pm_d.ap())
            nc.sync.dma_start(cm[:], cm_d.ap())
            nc.sync.dma_start(eye[:], eye_d.ap())

            # Three-stage software pipeline skewed across pairs so the
            # in-order ACT queue never blocks an early-stage pass behind a
            # late-stage pass of the previous pair.
            state = {}

            def stage_front(key):
                p = key % N_PAIRS
                x2 = io_pool.tile([128, 2 * W], bf16, tag="x2")
                nc.sync.dma_start(x2[:, 0:W], x_v[2 * p])
                nc.sync.dma_start(x2[:, W:2 * W], x_v[2 * p + 1])

                # pair-wide g PSUM (2 banks): one paired square pass on ACT
                g_ps = ps_g.tile([128, W], f32, tag="g")
                for h in (0, 1):
                    xh = x2[:, W * h:W * h + TILE_F]
                    xl = x2[:, W * h + TILE_F:W * (h + 1)]
                    half = g_ps[:, TILE_F * h:TILE_F * (h + 1)]
                    nc.tensor.matmul(half, bh[:], xh, start=True, stop=False)
                    nc.tensor.matmul(half, bh[:], xl, start=False, stop=False)
                    nc.tensor.matmul(half, bl[:], xh, start=False, stop=True)
                sq = work_pool.tile([128, W], f32, tag="sq")
                nc.scalar.square(sq[:], g_ps[:])

                # fused pair-sum + bias + transpose per 128-column chunk:
                # s_rm[f', 32g+k] = sum_j sq[32g+j, 128c+f'] * Pm[j,k] * c_k
                s_rm = ps_s.tile([128, W], f32, tag="s_rm")
                for cchunk in range(8):
                    nc.tensor.matmul(
                        s_rm[:, 128 * cchunk:128 * (cchunk + 1)],
                        sq[:, 128 * cchunk:128 * (cchunk + 1)],
                        pm[:],
                        start=True, stop=True,
                    )
                state[key] = {"s_rm": s_rm}

            def stage_midA(key):
                st = state[key]
                s_rm = st.pop("s_rm")
                mag_rm = work_pool.tile([128, W], f32, tag="mag_rm")
                nc.scalar.sqrt(mag_rm[:], s_rm[:])

                th8 = work_pool.tile([128, 8 * SEGS], f32, tag="th8")
                for t in range(SEGS):
                    nc.vector.max(
                        out=th8[:, 8 * t:8 * t + 8],
                        in_=mag_rm[:, 32 * t:32 * t + 32],
                    )
                st["mag_rm"] = mag_rm
                st["th8"] = th8

            def stage_midB(key):
                st = state[key]
                mag_rm = st.pop("mag_rm")
                th8 = st.pop("th8")
                th_b = th8[:, 7:8 * SEGS:8].to_broadcast([128, SEGS, 32])
                mag3 = mag_rm[:].rearrange("p (t n) -> p t n", n=32)

                mask = work_pool.tile([128, W], f32, tag="mask")
                mask3 = mask[:].rearrange("p (t n) -> p t n", n=32)
                # broadcast-AP operands are DVE-only (Pool rejects stride-0)
                nc.vector.tensor_tensor(
                    mask3, mag3, th_b, op=mybir.AluOpType.is_ge
                )

                coef_rm = work_pool.tile([128, W], bf16, tag="coef_rm")
                nc.gpsimd.tensor_mul(coef_rm[:], mask[:], mag_rm[:])
                st["coef_rm"] = coef_rm

            def stage_back(key):
                p = key % N_PAIRS
                st = state.pop(key)
                coef_rm = st["coef_rm"]
                # XBAR DMA transpose: all 8 [128,128] chunks in one
                # instruction, straight to SBUF (no PE pass, no ACT evac)
                coef = work_pool.tile([128, W], bf16, tag="coef")
                nc.sync.dma_start_transpose(
                    coef[:].rearrange("p (c f) -> p c f", c=8), coef_rm[:]
                )

                rec = ps_r.tile([128, W], f32, tag="rec")
                # one matmul per bank: a PSUM matmul output cannot cross banks
                nc.tensor.matmul(rec[:, 0:TILE_F], cm[:], coef[:, 0:TILE_F],
                                 start=True, stop=True)
                nc.tensor.matmul(rec[:, TILE_F:W], cm[:], coef[:, TILE_F:W],
                                 start=True, stop=True)

                o_sb = io_pool.tile([128, W], bf16, tag="o_sb")
                nc.scalar.copy(o_sb[:], rec[:])

                nc.sync.dma_start(out_v[2 * p], o_sb[:, 0:TILE_F])
                nc.sync.dma_start(out_v[2 * p + 1], o_sb[:, TILE_F:W])

            total = repeat * N_PAIRS
            for step in range(total + 3):
                if step < total:
                    stage_front(step)
                if 1 <= step <= total:
                    stage_midA(step - 1)
                if 2 <= step <= total + 1:
                    stage_midB(step - 2)
                if step >= 3:
                    stage_back(step - 3)

    nc.compile()
    return nc


def _get_program():
    if "nc" not in _cache:
        _cache["nc"] = _build_program()
    return _cache["nc"], _get_consts()


def _get_consts():
    if "consts" not in _cache:
        _cache["consts"] = _build_consts()
    return _cache["consts"]


def _pre_permute(xc: np.ndarray) -> np.ndarray:
    # [R_PER_CORE, 32] fp32 -> [N_TILES, 128, 1024] bf16 (hi || lo)
    import ml_dtypes

    t = xc.reshape(N_TILES, 4, TILE_F, S)          # [i, g, f, n]
    fm = np.ascontiguousarray(t.transpose(0, 1, 3, 2)).reshape(
        N_TILES, 128, TILE_F
    )
    hi = fm.astype(ml_dtypes.bfloat16)
    lo = (fm - hi.astype(np.float32)).astype(ml_dtypes.bfloat16)
    return np.concatenate([hi, lo], axis=2)


def _post_permute(op: np.ndarray) -> np.ndarray:
    # [N_TILES, 128, TILE_F] bf16 -> [R_PER_CORE, 32] fp32
    t = op.astype(np.float32).reshape(N_TILES, 4, S, TILE_F)
    t = t.transpose(0, 1, 3, 2)                    # [i, g, f, n]
    return np.ascontiguousarray(t).reshape(R_PER_CORE, S)


def _in_maps_from(xc: np.ndarray):
    bh, bl, pm, cm, eye = _get_consts()
    shards = xc.reshape(N_CORES, R_PER_CORE, S)
    return [
        {"x": _pre_permute(shards[c]), "Bh": bh, "Bl": bl, "Pm": pm,
         "Cm": cm, "Eye": eye}
        for c in range(N_CORES)
    ]


def _bench_in_maps():
    # Per-core input maps for the timing harness (test.py); mirrors kernel().
    rng = np.random.default_rng(0)
    xc = rng.standard_normal((B_TOTAL, S), dtype=np.float32)
    return _in_maps_from(xc)


def kernel(x: np.ndarray) -> np.ndarray:
    from concourse.bass_utils import run_bass_kernel_spmd

    nc, _ = _get_program()

    xc = np.ascontiguousarray(x[:, :, 0], dtype=np.float32)  # [B, 32]
    in_maps = _in_maps_from(xc)
    res = run_bass_kernel_spmd(nc, in_maps, core_ids=list(range(N_CORES)))
    out = np.concatenate(
        [_post_permute(r["out"]) for r in res.results], axis=0
    )
    return out.reshape(B_TOTAL, S, 1).astype(np.float32)
